# revision 49
# baseline (speedup 1.0000x reference)
"""Trainium2 Bass kernel for the CRAFT-style hard-negative-mining MSE loss.

Reference math (per branch, over N = 16*768*768 flat pixels):
    all_loss = (pred - target)^2
    pos_mask = (target >= 0.3) & (weight != 0)
    neg_mask = (target < 0.1)
    pos_sum  = sum(pos_mask * all_loss * weight)
    k        = min(max(1000, 3*num_pos), num_neg)
    topk_sum = sum of k largest all_loss among negatives
    loss     = (pos_sum + topk_sum) / (num_pos + k)
    out      = loss_char + loss_aff

With uniform targets num_pos ~ 0.7*N, so 3*num_pos >> num_neg and
k == num_neg: the top-k degenerates to the full sum over negatives.

Device strategy (TensorE-assisted): per 1/8 shard, per branch-tile
[128, W]:
    DVE:  d  = p - t                    tensor_tensor        (2x mode)
          m_neg = (t < 0.1)            tensor_scalar is_lt  (4x mode)
          m_pos = (t >= 0.3)           tensor_scalar is_ge  (4x mode)
          mw = m_pos * w                tensor_tensor        (2x mode)
    ACT:  l  = d^2                      Square (1x) -> strided [nb, 97]
    PE:   nb = W/96 blocks of 96 data cols:
            psum[0:97, 0:288] += [l_96 | 1]^T @ [m_neg | m_pos | mw]
          accumulated over the whole branch into one [97, 288] PSUM
          region:
            diag(rows 0:96 of group 0) -> per-col <m_neg, l> = S1
            diag(rows 0:96 of group 2) -> per-col <mw,    l> = S2
            row 96 of group 0          -> per-col sums of m_neg = num_neg
            row 96 of group 1          -> per-col sums of m_pos = num_pos
The [97, 288] PSUM regions are copied to SBUF (ScalarE) and DMA'd out
in 4 partition slices over 4 DMA queues; the host extracts
diagonals/count-rows, sums across the 8 shards, and applies the
k/denominator logic (with a full numpy fallback for the never-hit-here
k < num_neg case).

Tile widths are uneven on purpose: DVE per-op fixed overhead (~0.3us
of init bubble + drain per instruction) favors wide ops, but wide ops
at the pipeline ends serialize against DMA.  So tiles are small at the
very start (quick pipeline fill) and very end (short drain tail), and
wide in the DMA-saturated middle.

This moves the masked-sum and count reductions off DVE/ACT (where they
only run at 1x) onto the otherwise-idle TensorE.  Inputs are cast to
bf16 on the host: halves HBM traffic and doubles DVE tensor_tensor
throughput.
"""

import os
import numpy as np
import ml_dtypes

N_CORES = 8
B, H, W = 16, 768, 768
NPX = B * H * W              # 9_437_184 flat pixels
P = 128                      # SBUF partitions
FD = NPX // (N_CORES * P)    # 9216 free-dim elements per core per tensor
BD = 96                      # data columns per matmul block
PR = BD + 1                  # psum rows used (96 data + 1 count row)
WMAX = 2304                  # widest tile

# per-branch tile widths: small tiles at the global start (pipeline
# fill) and global end (drain tail) only
WIDTHS = [
    [1152, 1152, 2304, 2304, 2304],   # branch 0
    [2304, 2304, 2304, 1152, 1152],   # branch 1
]

THRESH_NEG = 0.1
THRESH_POS = 0.3

_compiled = None             # cached nc
LAST_RESULTS = None          # BassKernelResults of the last run (for profiling)


def _build_nc():
    import concourse.bacc as bacc
    import concourse.mybir as mybir
    import concourse.tile as tile
    from contextlib import ExitStack

    DT = mybir.dt.bfloat16
    f32 = mybir.dt.float32
    Alu = mybir.AluOpType
    Act = mybir.ActivationFunctionType

    nc = bacc.Bacc(
        "TRN2",
        target_bir_lowering=False,
        debug=False,
        num_devices=N_CORES,
    )

    # packed input: [P, branch, (p,t,w), FD]
    pk = nc.declare_dram_parameter("pk", [P, 2, 3, FD], DT, isOutput=False)
    out_ps = nc.declare_dram_parameter("acc_ps", [PR, 2, 3, BD], f32, isOutput=True)

    with tile.TileContext(nc) as tc, ExitStack() as ctx:
        in_pool = ctx.enter_context(tc.tile_pool(name="in", bufs=4))
        d_pool = ctx.enter_context(tc.tile_pool(name="d", bufs=2))
        m_pool = ctx.enter_context(tc.tile_pool(name="m", bufs=3))
        acc_pool = ctx.enter_context(tc.tile_pool(name="acc", bufs=1))
        ps_pool = ctx.enter_context(tc.psum_pool(name="ps", bufs=1))

        ps_sb = acc_pool.tile([PR, 2, 3, BD], f32, tag="ps_sb")
        psum = [
            ps_pool.tile([PR, 3, BD], f32, tag=f"psum{b}", name=f"psum{b}")
            for b in range(2)
        ]
        # persistent double-buffered [l | ones] stationary tensors; the
        # ones column (col 96 of each 97-block) is written once up front
        # and never touched again
        NBMAX = WMAX // BD
        lexts = [
            acc_pool.tile([P, NBMAX, PR], DT, tag=f"lext{j}", name=f"lext{j}")
            for j in range(2)
        ]
        for j in range(2):
            nc.gpsimd.memset(lexts[j][:, 0:3, 0:PR], 0.0)
            nc.gpsimd.memset(lexts[j][:, :, BD : BD + 1], 1.0)

        # PE warm-up: the HAM clock gate keeps TensorE at 1.2 GHz until it
        # has been busy ~3.4us.  The first real matmul only lands ~15us in
        # (after DMA+DVE+ACT of tile 0), so burn the idle head on dummy
        # matmuls into a scratch PSUM bank so the PE is at 2.4 GHz when
        # the real stream starts.
        ps_warm = ps_pool.tile([PR, 3 * PR], f32, tag="ps_warm")
        for _ in range(34):
            nc.tensor.matmul(
                ps_warm[:, :],
                lexts[0][:, 0, :],
                lexts[0][:, 0:3, :],
                start=True,
                stop=True,
            )

        it = 0
        for b in range(2):
            c0 = 0
            for i, Wt in enumerate(WIDTHS[b]):
                nb = Wt // BD
                sl_in = slice(c0, c0 + Wt)
                tin = in_pool.tile([P, 3, WMAX], DT, tag="in")
                nc.sync.dma_start(tin[:, :, 0:Wt], pk[:, b, :, sl_in])
                pt = tin[:, 0, 0:Wt]
                tt = tin[:, 1, 0:Wt]
                wt = tin[:, 2, 0:Wt]

                d = d_pool.tile([P, WMAX], DT, tag="d")
                lext = lexts[it % 2]
                m = m_pool.tile([P, 3, WMAX], DT, tag="m")
                # the very last tile is computed in two column halves so
                # the drain tail (last DMA -> last matmul) is shorter
                last = b == 1 and i == len(WIDTHS[b]) - 1
                halves = (
                    [(0, Wt // 2), (Wt // 2, Wt)] if last else [(0, Wt)]
                )
                for h0, h1 in halves:
                    # d = pred - target first, so ACT can start (DVE TT 2x)
                    nc.vector.tensor_tensor(
                        d[:, h0:h1], pt[:, h0:h1], tt[:, h0:h1], Alu.subtract
                    )
                    # l = d^2 into cols 0:96 of the 97-blocks (ACT Square 1x)
                    nc.scalar.activation(
                        lext[:, h0 // BD : h1 // BD, 0:BD],
                        d[:, h0:h1],
                        Act.Square,
                    )
                    # masks (DVE TS 4x): m[:,0]=(t<0.1)  m[:,1]=(t>=0.3)
                    nc.vector.tensor_scalar(
                        m[:, 0, h0:h1], tt[:, h0:h1], THRESH_NEG, None,
                        Alu.is_lt,
                    )
                    nc.vector.tensor_scalar(
                        m[:, 1, h0:h1], tt[:, h0:h1], THRESH_POS, None,
                        Alu.is_ge,
                    )
                    # m[:,2] = m_pos * w                      (DVE TT 2x)
                    nc.vector.tensor_tensor(
                        m[:, 2, h0:h1], m[:, 1, h0:h1], wt[:, h0:h1],
                        Alu.mult,
                    )

                # psum[b] += [l_blk | 1]^T @ [m_neg | m_pos | mw]   (PE)
                for k in range(nb):
                    sl = slice(k * BD, (k + 1) * BD)
                    nc.tensor.matmul(
                        psum[b][:, :, :],
                        lext[:, k, :],
                        m[:, :, sl],
                        start=(i == 0 and k == 0),
                        stop=(i == len(WIDTHS[b]) - 1 and k == nb - 1),
                    )
                c0 += Wt
                it += 1

            # dump the accumulated [97, 288] PSUM region to SBUF (ScalarE),
            # then DMA it out in 4 partition slices on 4 different engine
            # queues (a single contiguous store serializes on one DMA ring)
            nc.scalar.copy(ps_sb[:, b], psum[b][:, :, :])
            slices = [(0, 25), (25, 49), (49, 73), (73, PR)]
            issuers = [nc.sync, nc.gpsimd, nc.scalar, nc.gpsimd]
            for (p0, p1), eng in zip(slices, issuers):
                eng.dma_start(out_ps[p0:p1, b], ps_sb[p0:p1, b])

    nc.compile()
    return nc


def _get_nc():
    global _compiled
    if _compiled is None:
        _compiled = _build_nc()
    return _compiled


def _np_branch_fallback(pred, target, weight):
    """Exact reference math in numpy float64 (handles k < num_neg)."""
    pred = pred.astype(np.float64)
    target = target.astype(np.float64)
    weight = weight.astype(np.float64)
    all_loss = (pred - target) ** 2
    pos_mask = (target >= THRESH_POS) & (weight != 0)
    neg_mask = target < THRESH_NEG
    pos_sum = float(np.sum(np.where(pos_mask, all_loss * weight, 0.0)))
    num_pos = int(np.sum(pos_mask))
    num_neg = int(np.sum(neg_mask))
    k = min(max(1000, 3 * num_pos), num_neg)
    neg_vals = all_loss[neg_mask]
    if k >= num_neg:
        topk = float(neg_vals.sum())
    elif k <= 0:
        topk = 0.0
    else:
        topk = float(np.partition(neg_vals, num_neg - k)[num_neg - k :].sum())
    return (pos_sum + topk) / (num_pos + k)


def kernel(output, character_map, affinity_map, character_weight, affinity_weight):
    from concourse.bass_utils import run_bass_kernel_spmd

    global LAST_RESULTS
    np_dt = ml_dtypes.bfloat16

    output = np.asarray(output, dtype=np.float32)

    def shard(a):
        # flat pixel order (b, h, w) -> [core, partition, free]
        return np.ascontiguousarray(a).reshape(N_CORES, P, FD).astype(np_dt)

    packed = np.empty((N_CORES, P, 2, 3, FD), dtype=np_dt)
    packed[:, :, 0, 0] = shard(output[:, 0])
    packed[:, :, 0, 1] = shard(np.asarray(character_map, dtype=np.float32))
    packed[:, :, 0, 2] = shard(np.asarray(character_weight, dtype=np.float32))
    packed[:, :, 1, 0] = shard(output[:, 1])
    packed[:, :, 1, 1] = shard(np.asarray(affinity_map, dtype=np.float32))
    packed[:, :, 1, 2] = shard(np.asarray(affinity_weight, dtype=np.float32))

    in_maps = [{"pk": packed[c]} for c in range(N_CORES)]

    nc = _get_nc()
    res = run_bass_kernel_spmd(
        nc,
        in_maps,
        list(range(N_CORES)),
        trace=os.environ.get("KERNEL_TRACE", "0") == "1",
    )
    LAST_RESULTS = res

    # [cores, PR, branch, group, col]
    acc_ps = np.stack([r["acc_ps"] for r in res.results]).astype(np.float64)
    idx = np.arange(BD)
    s1 = acc_ps[:, idx, :, 0, idx].sum(axis=(0, 1))       # [branch]
    s2 = acc_ps[:, idx, :, 2, idx].sum(axis=(0, 1))       # [branch]
    n_neg = acc_ps[:, BD, :, 0, :].sum(axis=(0, 2))       # [branch]
    n_pos = acc_ps[:, BD, :, 1, :].sum(axis=(0, 2))       # [branch]

    total = 0.0
    for bidx, (tmap, wmap) in enumerate(
        [(character_map, character_weight), (affinity_map, affinity_weight)]
    ):
        num_neg = int(round(n_neg[bidx]))
        num_pos = int(round(n_pos[bidx]))
        k = min(max(1000, 3 * num_pos), num_neg)
        if k == num_neg:
            total += (s1[bidx] + s2[bidx]) / (num_pos + k)
        else:
            # top-k actually selective: fall back to exact host computation
            total += _np_branch_fallback(
                output[:, bidx].reshape(-1),
                np.asarray(tmap, dtype=np.float32).reshape(-1),
                np.asarray(wmap, dtype=np.float32).reshape(-1),
            )

    return np.float32(total)


# revision 52
# speedup vs baseline: 1.1101x; 1.1101x over previous
"""Trainium2 Bass kernel for the CRAFT-style hard-negative-mining MSE loss.

Reference math (per branch, over N = 16*768*768 flat pixels):
    all_loss = (pred - target)^2
    pos_mask = (target >= 0.3) & (weight != 0)
    neg_mask = (target < 0.1)
    pos_sum  = sum(pos_mask * all_loss * weight)
    k        = min(max(1000, 3*num_pos), num_neg)
    topk_sum = sum of k largest all_loss among negatives
    loss     = (pos_sum + topk_sum) / (num_pos + k)
    out      = loss_char + loss_aff

With uniform targets num_pos ~ 0.7*N, so 3*num_pos >> num_neg and
k == num_neg: the top-k degenerates to the full sum over negatives.

Device strategy (TensorE-assisted): per 1/8 shard, per branch-tile
[128, W]:
    DVE:  d  = p - t                    tensor_tensor        (2x mode)
          m_neg = (t < 0.1)            tensor_scalar is_lt  (4x mode)
          m_pos = (t >= 0.3)           tensor_scalar is_ge  (4x mode)
          mw = m_pos * w                tensor_tensor        (2x mode)
    ACT:  l  = d^2                      Square (1x) -> strided [nb, 97]
    PE:   nb = W/96 blocks of 96 data cols:
            psum[0:97, 0:288] += [l_96 | 1]^T @ [m_neg | m_pos | mw]
          accumulated over the whole branch into one [97, 288] PSUM
          region:
            diag(rows 0:96 of group 0) -> per-col <m_neg, l> = S1
            diag(rows 0:96 of group 2) -> per-col <mw,    l> = S2
            row 96 of group 0          -> per-col sums of m_neg = num_neg
            row 96 of group 1          -> per-col sums of m_pos = num_pos
The [97, 288] PSUM regions are copied to SBUF (ScalarE) and DMA'd out
in 4 partition slices over 4 DMA queues; the host extracts
diagonals/count-rows, sums across the 8 shards, and applies the
k/denominator logic (with a full numpy fallback for the never-hit-here
k < num_neg case).

Tile widths are uneven on purpose: DVE per-op fixed overhead (~0.3us
of init bubble + drain per instruction) favors wide ops, but wide ops
at the pipeline ends serialize against DMA.  So tiles are small at the
very start (quick pipeline fill) and very end (short drain tail), and
wide in the DMA-saturated middle.

This moves the masked-sum and count reductions off DVE/ACT (where they
only run at 1x) onto the otherwise-idle TensorE.  Inputs are cast to
bf16 on the host: halves HBM traffic and doubles DVE tensor_tensor
throughput.
"""

import os
import numpy as np
import ml_dtypes

N_CORES = 8
B, H, W = 16, 768, 768
NPX = B * H * W              # 9_437_184 flat pixels
P = 128                      # SBUF partitions
FD = NPX // (N_CORES * P)    # 9216 free-dim elements per core per tensor
BD = 96                      # data columns per matmul block
PR = BD + 1                  # psum rows used (96 data + 1 count row)
WMAX = 2304                  # widest tile

# per-branch tile widths: small tiles at the global start (pipeline
# fill) and global end (drain tail) only
WIDTHS = [
    [1152, 1152, 2304, 2304, 2304],   # branch 0
    [2304, 2304, 2304, 1152, 1152],   # branch 1
]

THRESH_NEG = 0.1
THRESH_POS = 0.3

_compiled = None             # cached nc
LAST_RESULTS = None          # BassKernelResults of the last run (for profiling)


def _build_nc():
    import concourse.bacc as bacc
    import concourse.mybir as mybir
    import concourse.tile as tile
    from contextlib import ExitStack

    DT = mybir.dt.bfloat16
    f32 = mybir.dt.float32
    Alu = mybir.AluOpType
    Act = mybir.ActivationFunctionType

    nc = bacc.Bacc(
        "TRN2",
        target_bir_lowering=False,
        debug=False,
        num_devices=N_CORES,
    )

    # packed input: [P, branch, (p,t,w), FD]
    pk = nc.declare_dram_parameter("pk", [P, 2, 3, FD], DT, isOutput=False)
    out_ps = nc.declare_dram_parameter("acc_ps", [PR, 2, 3, BD], f32, isOutput=True)

    with tile.TileContext(nc) as tc, ExitStack() as ctx:
        in_pool = ctx.enter_context(tc.tile_pool(name="in", bufs=4))
        d_pool = ctx.enter_context(tc.tile_pool(name="d", bufs=2))
        m_pool = ctx.enter_context(tc.tile_pool(name="m", bufs=3))
        acc_pool = ctx.enter_context(tc.tile_pool(name="acc", bufs=1))
        ps_pool = ctx.enter_context(tc.psum_pool(name="ps", bufs=1))

        ps_sb = acc_pool.tile([PR, 2, 3, BD], f32, tag="ps_sb")
        psum = [
            ps_pool.tile([PR, 3, BD], f32, tag=f"psum{b}", name=f"psum{b}")
            for b in range(2)
        ]
        # persistent double-buffered [l | ones] stationary tensors; the
        # ones column (col 96 of each 97-block) is written once up front
        # and never touched again
        NBMAX = WMAX // BD
        lexts = [
            acc_pool.tile([P, NBMAX, PR], DT, tag=f"lext{j}", name=f"lext{j}")
            for j in range(2)
        ]
        for j in range(2):
            nc.gpsimd.memset(lexts[j][:, 0:3, 0:PR], 0.0)
            nc.gpsimd.memset(lexts[j][:, :, BD : BD + 1], 1.0)

        # PE warm-up: the HAM clock gate keeps TensorE at 1.2 GHz until it
        # has been busy ~3.4us.  The first real matmul only lands ~15us in
        # (after DMA+DVE+ACT of tile 0), so burn the idle head on dummy
        # matmuls into a scratch PSUM bank so the PE is at 2.4 GHz when
        # the real stream starts.
        ps_warm = ps_pool.tile([PR, 3 * PR], f32, tag="ps_warm")
        for _ in range(34):
            nc.tensor.matmul(
                ps_warm[:, :],
                lexts[0][:, 0, :],
                lexts[0][:, 0:3, :],
                start=True,
                stop=True,
            )

        it = 0
        for b in range(2):
            c0 = 0
            for i, Wt in enumerate(WIDTHS[b]):
                nb = Wt // BD
                sl_in = slice(c0, c0 + Wt)
                tin = in_pool.tile([P, 3, WMAX], DT, tag="in")
                nc.sync.dma_start(tin[:, :, 0:Wt], pk[:, b, :, sl_in])
                pt = tin[:, 0, 0:Wt]
                tt = tin[:, 1, 0:Wt]
                wt = tin[:, 2, 0:Wt]

                d = d_pool.tile([P, WMAX], DT, tag="d")
                lext = lexts[it % 2]
                m = m_pool.tile([P, 3, WMAX], DT, tag="m")
                # the very last tile is computed in two column halves so
                # the drain tail (last DMA -> last matmul) is shorter
                last = b == 1 and i == len(WIDTHS[b]) - 1
                halves = (
                    [(0, Wt // 2), (Wt // 2, Wt)] if last else [(0, Wt)]
                )
                for h0, h1 in halves:
                    # d = pred - target first, so ACT can start (DVE TT 2x)
                    nc.vector.tensor_tensor(
                        d[:, h0:h1], pt[:, h0:h1], tt[:, h0:h1], Alu.subtract
                    )
                    # l = d^2 into cols 0:96 of the 97-blocks (ACT Square 1x)
                    nc.scalar.activation(
                        lext[:, h0 // BD : h1 // BD, 0:BD],
                        d[:, h0:h1],
                        Act.Square,
                    )
                    # masks (DVE TS 4x): m[:,0]=(t<0.1)  m[:,1]=(t>=0.3)
                    nc.vector.tensor_scalar(
                        m[:, 0, h0:h1], tt[:, h0:h1], THRESH_NEG, None,
                        Alu.is_lt,
                    )
                    nc.vector.tensor_scalar(
                        m[:, 1, h0:h1], tt[:, h0:h1], THRESH_POS, None,
                        Alu.is_ge,
                    )
                    # m[:,2] = m_pos * w                      (DVE TT 2x)
                    nc.vector.tensor_tensor(
                        m[:, 2, h0:h1], m[:, 1, h0:h1], wt[:, h0:h1],
                        Alu.mult,
                    )

                # psum[b] += [l_blk | 1]^T @ [m_neg | m_pos | mw]   (PE)
                for k in range(nb):
                    sl = slice(k * BD, (k + 1) * BD)
                    nc.tensor.matmul(
                        psum[b][:, :, :],
                        lext[:, k, :],
                        m[:, :, sl],
                        start=(i == 0 and k == 0),
                        stop=(i == len(WIDTHS[b]) - 1 and k == nb - 1),
                    )
                c0 += Wt
                it += 1

            # dump the accumulated [97, 288] PSUM region to SBUF (ScalarE),
            # then DMA it out in 4 partition slices on 4 different engine
            # queues (a single contiguous store serializes on one DMA ring)
            nc.scalar.copy(ps_sb[:, b], psum[b][:, :, :])
            slices = [(0, 25), (25, 49), (49, 73), (73, PR)]
            issuers = [nc.sync, nc.gpsimd, nc.scalar, nc.gpsimd]
            for (p0, p1), eng in zip(slices, issuers):
                eng.dma_start(out_ps[p0:p1, b], ps_sb[p0:p1, b])

    nc.compile()
    return nc


def _get_nc():
    global _compiled
    if _compiled is None:
        _compiled = _build_nc()
    return _compiled


def _np_branch_fallback(pred, target, weight):
    """Exact reference math in numpy float64 (handles k < num_neg)."""
    pred = pred.astype(np.float64)
    target = target.astype(np.float64)
    weight = weight.astype(np.float64)
    all_loss = (pred - target) ** 2
    pos_mask = (target >= THRESH_POS) & (weight != 0)
    neg_mask = target < THRESH_NEG
    pos_sum = float(np.sum(np.where(pos_mask, all_loss * weight, 0.0)))
    num_pos = int(np.sum(pos_mask))
    num_neg = int(np.sum(neg_mask))
    k = min(max(1000, 3 * num_pos), num_neg)
    neg_vals = all_loss[neg_mask]
    if k >= num_neg:
        topk = float(neg_vals.sum())
    elif k <= 0:
        topk = 0.0
    else:
        topk = float(np.partition(neg_vals, num_neg - k)[num_neg - k :].sum())
    return (pos_sum + topk) / (num_pos + k)


def kernel(output, character_map, affinity_map, character_weight, affinity_weight):
    from concourse.bass_utils import run_bass_kernel_spmd

    global LAST_RESULTS
    np_dt = ml_dtypes.bfloat16

    output = np.asarray(output, dtype=np.float32)

    def shard(a):
        # flat pixel order (b, h, w) -> [core, partition, free]
        return np.ascontiguousarray(a).reshape(N_CORES, P, FD).astype(np_dt)

    packed = np.empty((N_CORES, P, 2, 3, FD), dtype=np_dt)
    packed[:, :, 0, 0] = shard(output[:, 0])
    packed[:, :, 0, 1] = shard(np.asarray(character_map, dtype=np.float32))
    packed[:, :, 0, 2] = shard(np.asarray(character_weight, dtype=np.float32))
    packed[:, :, 1, 0] = shard(output[:, 1])
    packed[:, :, 1, 1] = shard(np.asarray(affinity_map, dtype=np.float32))
    packed[:, :, 1, 2] = shard(np.asarray(affinity_weight, dtype=np.float32))

    in_maps = [{"pk": packed[c]} for c in range(N_CORES)]

    nc = _get_nc()
    res = run_bass_kernel_spmd(
        nc,
        in_maps,
        list(range(N_CORES)),
        trace=os.environ.get("KERNEL_TRACE", "0") == "1",
    )
    LAST_RESULTS = res

    # [cores, PR, branch, group, col]
    acc_ps = np.stack([r["acc_ps"] for r in res.results]).astype(np.float64)
    idx = np.arange(BD)
    s1 = acc_ps[:, idx, :, 0, idx].sum(axis=(0, 1))       # [branch]
    s2 = acc_ps[:, idx, :, 2, idx].sum(axis=(0, 1))       # [branch]
    n_neg = acc_ps[:, BD, :, 0, :].sum(axis=(0, 2))       # [branch]
    n_pos = acc_ps[:, BD, :, 1, :].sum(axis=(0, 2))       # [branch]

    total = 0.0
    for bidx, (tmap, wmap) in enumerate(
        [(character_map, character_weight), (affinity_map, affinity_weight)]
    ):
        num_neg = int(round(n_neg[bidx]))
        num_pos = int(round(n_pos[bidx]))
        k = min(max(1000, 3 * num_pos), num_neg)
        if k == num_neg:
            total += (s1[bidx] + s2[bidx]) / (num_pos + k)
        else:
            # top-k actually selective: fall back to exact host computation
            total += _np_branch_fallback(
                output[:, bidx].reshape(-1),
                np.asarray(tmap, dtype=np.float32).reshape(-1),
                np.asarray(wmap, dtype=np.float32).reshape(-1),
            )

    return np.float32(total)
